# revision 1
# baseline (speedup 1.0000x reference)
"""BasicTransformerBlock on 8 TRN2 NeuronCores.

Sharding: data-parallel, core = (batch b in 0..3) x (sequence half h in 0..1).
Each core receives its batch element's full sequence rotated so its local 512
rows come first (softmax over keys is permutation invariant), computes K/V of
attn1 for all 1024 tokens (duplicated across the pair, zero collectives), and
everything else for its 512 local tokens only.

On-chip layout: feature-major activations [features on partitions, tokens on
free axis]. The attention projections (Q/K/V/O of both attns) and attnV of
attn1 run in fp8e4 with DoubleRow perf mode (256-row contraction per matmul,
~2x PE throughput); weights are pre-scaled by a power of two on the host and
descaled at PSUM evacuation. The GEGLU FF stays fp16 (fp8 there costs too
much accuracy). LayerNorm reductions use ones-matmuls; rstd is computed as
exp(-0.5*ln(var+eps)) so the whole kernel (except the final gelu) uses a
single ACT table set. Softmax denominators come free from a ones-column
appended to V.
"""

import sys
import types

sys.path.insert(0, "/opt/trn_rl_repo")

# concourse fetches the NTFF profile hook from antenv.axon_hooks, which the
# agent image's antenv stub lacks. Register a shim so trace=True works.
if "antenv.axon_hooks" not in sys.modules:
    _hooks = types.ModuleType("antenv.axon_hooks")
    _HOOK = [None]

    def _get_hook():
        if _HOOK[0] is None:
            try:
                from trn_agent_boot.trn_boot import _ntff_profile_via_ctypes

                _HOOK[0] = _ntff_profile_via_ctypes("/opt/axon/libaxon_pjrt.so")
            except Exception:
                _HOOK[0] = None
        return _HOOK[0]

    _hooks.get_axon_ntff_profile_hook = _get_hook
    _hooks.set_axon_ntff_profile_hook = lambda h: _HOOK.__setitem__(0, h)
    sys.modules["antenv.axon_hooks"] = _hooks
    try:
        import antenv

        antenv.axon_hooks = _hooks
    except ImportError:
        pass

import os
import numpy as np
import ml_dtypes

import concourse.bass as bass
import concourse.mybir as mybir
import concourse.tile as tile
from concourse import bacc, bass_utils

dt = mybir.dt
F32, F16, F8 = dt.float32, dt.float16, dt.float8e4
E4 = ml_dtypes.float8_e4m3
AF = mybir.ActivationFunctionType
DR = mybir.MatmulPerfMode.DoubleRow
ALU = mybir.AluOpType

DIM, HEADS, DHEAD, CTX_DIM, DFF = 1280, 20, 64, 768, 5120
BATCH, NTOK, MCTX = 4, 1024, 77
EPS = 1e-5
SCALE = DHEAD ** -0.5
N_CORES = 8
T = 512         # local tokens per core
TKV = 1024      # attn1 key/value tokens per core
KC = DIM // 128           # 10
K4 = KC // 2              # 5 double-row chunks
KCX = CTX_DIM // 128      # 6
JFF = DFF // 128          # 40
P = 128

last_exec_time_ns = None
last_results = None


KDBG = bool(os.environ.get("KDBG"))


def _emit(tc, d, trivial_aff, trivial_bias, ws_inv):
    nc = tc.nc
    pools = {}

    def pool(name, bufs, space="SBUF", side="left"):
        p = tc.alloc_tile_pool(name=name, bufs=bufs, space=space, side=side)
        pools[name] = p
        return p

    def close(*names):
        for n in names:
            pools.pop(n).release()

    # Pools are two LIFO stacks (left/right) per memory space; lifetimes below
    # are arranged so every release pops the top of its stack.
    const = pool("const", 1)
    ones_col = const.tile([P, 1], F16, name="ones_col")
    nc.vector.memset(ones_col[:], 1.0)
    ones_row = const.tile([1, P], F16, name="ones_row")
    nc.vector.memset(ones_row[:], 1.0)
    # selector for per-pair reciprocal broadcast: out[p,:] = rec[32*(p//64),:]
    # (head lanes live at partitions 0 and 32 -- partition starts must be
    # 32-aligned for engine access patterns)
    sel2 = const.tile([33, P], F16, name="sel2")
    nc.vector.memset(sel2[:], 0.0)
    nc.vector.memset(sel2[0:1, 0:64], 1.0)
    nc.vector.memset(sel2[32:33, 64:128], 1.0)
    eps_ap = const.tile([1, 1], F32, name="eps_ap")
    nc.vector.memset(eps_ap[:], EPS)
    if not trivial_aff:
        aff = const.tile([P, 60], F32, name="aff")
        nc.sync.dma_start(aff[:], d["aff"])
    if not trivial_bias:
        biases = const.tile([P, 110], F32, name="biases")
        nc.sync.dma_start(biases[:], d["biases"])

    tmp = pool("tmp", 1)

    # ---------------- helpers ----------------

    def layernorm(x_src, Ttok, ln_idx, out_big, ln_psum, xr32=False):
        """x_src: list of KC tiles/APs [128, Ttok] f32.
        out_big: [128, KC, Ttok] tile (f8 or f16)."""
        bcasts = []
        for g in range(Ttok // 512):
            sl = slice(g * 512, (g + 1) * 512)
            sums_ps = ln_psum.tile([1, 512], F32, name=f"lns{ln_idx}_{g}", tag="lnstat",
                                   bufs=2)
            sq_ps = ln_psum.tile([1, 512], F32, name=f"lnq{ln_idx}_{g}", tag="lnstat",
                                 bufs=2)
            for c in range(KC):
                xh = tmp.tile([P, 512], F16, name=f"xh{ln_idx}_{g}_{c}", tag="xh",
                              bufs=4)
                nc.vector.tensor_copy(out=xh[:], in_=x_src[c][:, sl])
                xsq = tmp.tile([P, 512], F16, name=f"xq{ln_idx}_{g}_{c}", tag="xsq",
                               bufs=4)
                nc.gpsimd.tensor_mul(xsq[:], xh[:], xh[:])
                nc.tensor.matmul(sums_ps[:], ones_col[:], xh[:],
                                 start=(c == 0), stop=(c == KC - 1))
                nc.tensor.matmul(sq_ps[:], ones_col[:], xsq[:],
                                 start=(c == 0), stop=(c == KC - 1))
            mu = tmp.tile([1, 512], F32, name=f"mu{ln_idx}_{g}", tag="mu", bufs=1)
            nc.vector.tensor_scalar_mul(mu[:], sums_ps[:], 1.0 / DIM)
            musq = tmp.tile([1, 512], F32, name=f"musq{ln_idx}_{g}", tag="musq", bufs=1)
            nc.vector.tensor_mul(musq[:], mu[:], mu[:])
            var = tmp.tile([1, 512], F32, name=f"var{ln_idx}_{g}", tag="var", bufs=1)
            # var = sq/DIM - mu^2  (EPS added via Ln bias)
            nc.vector.scalar_tensor_tensor(var[:], sq_ps[:], 1.0 / DIM, musq[:],
                                           ALU.mult, ALU.subtract)
            lnv = tmp.tile([1, 512], F32, name=f"lnv{ln_idx}_{g}", tag="lnv", bufs=1)
            nc.scalar.activation(lnv[:], var[:], AF.Ln, bias=eps_ap[:])
            rstd = tmp.tile([1, 512], F32, name=f"rst{ln_idx}_{g}", tag="rstd", bufs=1)
            nc.scalar.activation(rstd[:], lnv[:], AF.Exp, scale=-0.5)
            rstd16 = tmp.tile([1, 512], F16, name=f"rs16{ln_idx}_{g}", tag="rstd16", bufs=1)
            nc.vector.tensor_copy(out=rstd16[:], in_=rstd[:])
            murstd16 = tmp.tile([1, 512], F16, name=f"mr16{ln_idx}_{g}", tag="mr16", bufs=1)
            nc.vector.tensor_mul(murstd16[:], mu[:], rstd[:])
            rB_ps = ln_psum.tile([P, 512], F32, name=f"rb{ln_idx}_{g}", tag="lnbc",
                                 bufs=2)
            nc.tensor.matmul(rB_ps[:], ones_row[:], rstd16[:], start=True, stop=True)
            mB_ps = ln_psum.tile([P, 512], F32, name=f"mb{ln_idx}_{g}", tag="lnbc",
                                 bufs=2)
            nc.tensor.matmul(mB_ps[:], ones_row[:], murstd16[:], start=True, stop=True)
            rB = tmp.tile([P, 512], F16, name=f"rB{ln_idx}_{g}", tag="rB", bufs=2)
            nc.vector.tensor_copy(out=rB[:], in_=rB_ps[:])
            mB = tmp.tile([P, 512], F16, name=f"mB{ln_idx}_{g}", tag="mB", bufs=2)
            nc.vector.tensor_copy(out=mB[:], in_=mB_ps[:])
            bcasts.append((sl, rB, mB))
        for g, (sl, rB, mB) in enumerate(bcasts):
            for c in range(KC):
                eng = nc.gpsimd if c % 2 == 0 else nc.vector
                xr = tmp.tile([P, 512], F16, name=f"xr{ln_idx}_{g}_{c}", tag="xr",
                              bufs=6)
                eng.tensor_mul(xr[:], x_src[c][:, sl], rB[:])
                if trivial_aff:
                    eng.tensor_sub(out_big[:, c, sl], xr[:], mB[:])
                else:
                    xn = tmp.tile([P, 512], F16, name=f"xn{ln_idx}_{g}_{c}", tag="xn",
                                  bufs=6)
                    eng.tensor_sub(xn[:], xr[:], mB[:])
                    g_ap = aff[:, ln_idx * 20 + c: ln_idx * 20 + c + 1]
                    be_ap = aff[:, ln_idx * 20 + 10 + c: ln_idx * 20 + 10 + c + 1]
                    eng.tensor_scalar(out_big[:, c, sl], xn[:], g_ap, be_ap,
                                      ALU.mult, ALU.add)

    def project_dr(w_d, rhs_fn, n_mc, consume, wpool, wtag, psum_p, wbufs=3):
        """out[mc] = sum_k4 w[mc][:, k4].T @@DR rhs(k4); consume(mc, psum)."""
        for mc in range(n_mc):
            wt = wpool.tile([P, K4, 2, P], F8, name=f"{wtag}_{mc}", tag=wtag,
                            bufs=wbufs)
            nc.sync.dma_start(wt[:], w_d[mc])
            ps = psum_p.tile([P, 512], F32, name=f"ps_{wtag}_{mc}", tag="proj", bufs=4)
            for k4 in range(K4):
                nc.tensor.matmul(ps[:], wt[:, k4], rhs_fn(k4),
                                 start=(k4 == 0), stop=(k4 == K4 - 1), perf_mode=DR)
            consume(mc, ps)

    def bias_ap(col):
        return biases[:, col:col + 1]

    # ---------------- phase 1: load x, LN1 ----------------

    ln1p = pool("ln1p", 1)
    ln1 = ln1p.tile([P, KC, TKV], F8, name="ln1")

    ln_psum = pool("ln_psum", 1, space="PSUM")
    xpool = pool("xpool", 1, side="right")
    x_sb = []
    for c in range(KC):
        xc = xpool.tile([P, TKV], F32, name=f"x_{c}", tag="x", bufs=KC)
        nc.sync.dma_start(xc[:], d["xt"][c * P:(c + 1) * P, :])
        x_sb.append(xc)
    layernorm(x_sb, TKV, 0, ln1, ln_psum, xr32=True)
    if KDBG:
        nc.sync.dma_start(d["dbg_ln1"], ln1[:])
    close("xpool", "ln_psum")

    # ---------------- phase 2: Q1/K1 (fp8 DR) + ctx/K2/V2 (fp16) ----------------

    proj_psum = pool("proj_psum", 1, space="PSUM")
    x2_psum = pool("x2_psum", 1, space="PSUM")
    wpool = pool("wpool1", 1)
    qkv2 = pool("qkv2", 1, side="right")
    qkv = pool("qkv", 1, side="right")

    Qt = [qkv.tile([P, T], F16, name=f"qt_{mc}", tag="qt", bufs=KC) for mc in range(KC)]
    Kt = [qkv.tile([P, TKV], F16, name=f"kt_{mc}", tag="kt", bufs=KC)
          for mc in range(KC)]

    def q_consume(mc, ps):
        if mc % 2 == 0:
            nc.scalar.mul(Qt[mc][:], ps[:], ws_inv)
        else:
            nc.vector.tensor_scalar_mul(Qt[mc][:], ps[:], ws_inv)

    project_dr(d["wq1"], lambda k4: ln1[:, 2 * k4:2 * k4 + 2, 0:T], KC, q_consume,
               wpool, "wq1", proj_psum)

    for thalf in range(2):
        sl = slice(thalf * 512, (thalf + 1) * 512)

        def k_consume(mc, ps, sl=sl):
            if mc % 2 == 0:
                nc.scalar.mul(Kt[mc][:, sl], ps[:], ws_inv)
            else:
                nc.vector.tensor_scalar_mul(Kt[mc][:, sl], ps[:], ws_inv)

        project_dr(d["wk1"], lambda k4, sl=sl: ln1[:, 2 * k4:2 * k4 + 2, sl], KC,
                   k_consume, wpool, "wk1", proj_psum)

    if KDBG:
        for mc in range(KC):
            nc.sync.dma_start(d["dbg_q"][mc], Qt[mc][:])
            nc.sync.dma_start(d["dbg_k"][mc], Kt[mc][:])

    # ctx load + K2 + V2 (small fp16 work, independent of attn1)
    wpool0 = pool("wpool0", 1)
    ctx_sb = []
    for c in range(KCX):
        cc = tmp.tile([P, MCTX], F32, name=f"ctx_{c}", tag="ctx", bufs=KCX)
        nc.sync.dma_start(cc[:], d["ctxt"][c * P:(c + 1) * P, :])
        ch = qkv2.tile([P, MCTX], F16, name=f"ctxh_{c}", tag="ctxh", bufs=KCX)
        nc.vector.tensor_copy(out=ch[:], in_=cc[:])
        ctx_sb.append(ch)

    K2t = [qkv2.tile([P, MCTX], F16, name=f"k2t_{mc}", tag="k2t", bufs=KC)
           for mc in range(KC)]
    V2t = qkv2.tile([P, HEADS, DHEAD + 1], F16, name="v2t")

    for mc in range(KC):
        wt = wpool0.tile([P, KCX, P], F16, name=f"wk2_{mc}", tag="wk2", bufs=2)
        nc.sync.dma_start(wt[:], d["wk2"][mc])
        ps = x2_psum.tile([P, MCTX], F32, name=f"psk2_{mc}", tag="projx", bufs=2)
        for kc in range(KCX):
            nc.tensor.matmul(ps[:], wt[:, kc], ctx_sb[kc][:], start=(kc == 0),
                             stop=(kc == KCX - 1))
        nc.vector.tensor_copy(out=K2t[mc][:], in_=ps[:])

    nc.vector.memset(V2t[:], 1.0)
    wv2_sb = []
    for kc in range(KCX):
        wv = wpool0.tile([P, DIM], F16, name=f"wv2_{kc}", tag="wv2", bufs=KCX)
        nc.sync.dma_start(wv[:], d["wv2"][kc])
        wv2_sb.append(wv)
    for n0, nsz in ((0, 512), (512, 512), (1024, 256)):
        ps = x2_psum.tile([MCTX, 512], F32, name=f"psv2_{n0}", tag="projx2", bufs=2)
        for kc in range(KCX):
            nc.tensor.matmul(ps[:, :nsz], ctx_sb[kc][:], wv2_sb[kc][:, n0:n0 + nsz],
                             start=(kc == 0), stop=(kc == KCX - 1))
        nc.vector.tensor_copy(
            out=V2t[:MCTX, n0 // DHEAD:(n0 + nsz) // DHEAD, 0:DHEAD],
            in_=ps[:, :nsz].rearrange("p (h e) -> p h e", e=DHEAD))

    close("wpool0", "x2_psum", "proj_psum")

    # ---------------- phase 3: attn1 (V1 DR projection runs as filler) --------

    otp = pool("otp", 1)
    Ot = otp.tile([P, KC, T], F8, name="ot")

    sc_psum = pool("sc_psum", 1, space="PSUM")
    vp_psum = pool("vp_psum", 1, space="PSUM")
    ov_psum = pool("ov_psum", 1, space="PSUM")
    bp_psum = pool("bp_psum", 1, space="PSUM")
    vtp = pool("vtp", 1, side="right")
    epool = pool("epool", 12, side="right")

    Vt = [vtp.tile([P, 2, HEADS, 80], F8, name=f"vt_{k4}", tag="vt", bufs=4)
          for k4 in range(4)]
    for k4 in range(4):
        nc.vector.memset(Vt[k4][:, :, :, 64:65], 1.0)

    eps_attn = {}  # (c, k4g) -> exp tile

    def vproj_filler(nt):
        n0, nsz = ((0, 512), (512, 512), (1024, 256))[nt]

        def run():
            wv_sl = []
            for k4 in range(K4):
                wv = wpool.tile([P, 2, 512], F8, name=f"wv1_{nt}_{k4}", tag="wv1",
                                bufs=K4)
                nc.sync.dma_start(wv[:, :, :nsz], d["wv1"][k4][:, :, n0:n0 + nsz])
                wv_sl.append(wv)
            for t8 in range(8):
                ps = vp_psum.tile([P, 512], F32, name=f"psv_{t8}_{n0}", tag="vproj",
                                  bufs=1)
                for k4 in range(K4):
                    nc.tensor.matmul(ps[:, :nsz],
                                     ln1[:, 2 * k4:2 * k4 + 2, t8 * P:(t8 + 1) * P],
                                     wv_sl[k4][:, :, :nsz],
                                     start=(k4 == 0), stop=(k4 == K4 - 1),
                                     perf_mode=DR)
                nc.vector.tensor_scalar_mul(
                    Vt[t8 // 2][:, t8 % 2, n0 // DHEAD:(n0 + nsz) // DHEAD, 0:DHEAD],
                    ps[:, :nsz].rearrange("p (h e) -> p h e", e=DHEAD), ws_inv)
        return run

    fillers = (vproj_filler(0), vproj_filler(1), vproj_filler(2))

    def attn1_scores(c, k4g):
        ep = epool.tile([P, 2, 1024], F8, name=f"exp{c}_{k4g}", tag="exp")
        for kk in range(2):
            k8 = 2 * k4g + kk
            sps = sc_psum.tile([P, 1024], F32, name=f"sps{c}_{k8}", tag="sc", bufs=2)
            for h in range(2):
                nc.tensor.matmul(sps[:, h * 512: h * 512 + 512],
                                 Kt[c][64 * h:64 * h + 64, k8 * P:(k8 + 1) * P],
                                 Qt[c][64 * h:64 * h + 64, :],
                                 start=True, stop=True, tile_position=(64 * h, 0))
            nc.scalar.activation(ep[:, kk, :], sps[:], AF.Exp, scale=SCALE)
        eps_attn[(c, k4g)] = ep

    def attn1_av(c, k4g, ov):
        for h in range(2):
            nc.tensor.matmul(ov[h][:], Vt[k4g][:, :, 2 * c + h, 0:DHEAD + 1],
                             eps_attn[(c, k4g)][:, :, h * 512:(h + 1) * 512],
                             start=(k4g == 0), stop=(k4g == 3), perf_mode=DR)

    def finish(c, ov, O_big):
        usbp = tmp.tile([P, 512], F16, name=f"usb{c}", tag="usb", bufs=3)
        dens = [tmp.tile([1, 512], F32, name=f"den{c}_{h}", tag=f"den{h}", bufs=2)
                for h in range(2)]
        for h in range(2):
            nc.vector.tensor_copy(out=usbp[64 * h:64 * h + 64, :], in_=ov[h][0:64, :])
            nc.vector.tensor_copy(out=dens[h][:], in_=ov[h][64:65, :])
        reca = tmp.tile([1, 512], F32, name=f"rarc16{c}", tag="reca", bufs=2)
        recb = tmp.tile([1, 512], F32, name=f"rbrc16{c}", tag="recb", bufs=2)
        nc.vector.reciprocal_approx_fast(reca[:], dens[0][:])
        nc.vector.reciprocal_approx_fast(recb[:], dens[1][:])
        rec16 = tmp.tile([33, 512], F16, name=f"rc16{c}", tag="rec16", bufs=3)
        nc.vector.memset(rec16[:], 0.0)
        nc.vector.tensor_copy(out=rec16[0:1, :], in_=reca[:])
        nc.vector.tensor_copy(out=rec16[32:33, :], in_=recb[:])
        bps = bp_psum.tile([P, 512], F32, name=f"bps{c}", tag="bps", bufs=1)
        nc.tensor.matmul(bps[:], sel2[:], rec16[:], start=True, stop=True)
        nc.vector.tensor_mul(O_big[:, c, :], usbp[:], bps[:])
        if KDBG and c == 0:
            nc.sync.dma_start(d["dbg_usb"], usbp[:])
            nc.sync.dma_start(d["dbg_rec"], rec16[:])

    def alloc_ov(c):
        return [ov_psum.tile([DHEAD + 1, 512], F32, name=f"ov{2 * c + h}", tag="ov",
                             bufs=2) for h in range(2)]

    LAG = 2
    pend = []
    for c in range(KC):
        dc = dov = None
        if len(pend) >= LAG:
            dc = pend.pop(0)
            dov = alloc_ov(dc)
        for k4g in range(4):
            attn1_scores(c, k4g)
            if dov is not None:
                attn1_av(dc, k4g, dov)
        if dov is not None:
            finish(dc, dov, Ot)
        if c < len(fillers):
            fillers[c]()
        pend.append(c)
    for dc in pend:
        dov = alloc_ov(dc)
        for k4g in range(4):
            attn1_av(dc, k4g, dov)
        finish(dc, dov, Ot)

    close("epool", "vtp", "qkv", "bp_psum", "ov_psum", "vp_psum", "sc_psum")

    # ---------------- phase 4: out-proj 1 + residual, LN2, Q2 ----------------

    resp = pool("resp", 1)
    resid = []
    for c in range(KC):
        rc = resp.tile([P, T], F32, name=f"res_{c}", tag="res", bufs=KC)
        nc.sync.dma_start(rc[:], d["xres"][c * P:(c + 1) * P, :])
        resid.append(rc)

    proj_psum = pool("proj_psum2", 1, space="PSUM")
    x1p = pool("x1p", 1, side="right")
    x1 = [x1p.tile([P, T], F32, name=f"x1_{mc}", tag="x1", bufs=KC) for mc in range(KC)]

    def o1_consume(mc, ps):
        nc.vector.scalar_tensor_tensor(x1[mc][:], ps[:], ws_inv, resid[mc][:],
                                       ALU.mult, ALU.add)
        if not trivial_bias:
            nc.vector.tensor_scalar_add(x1[mc][:], x1[mc][:], bias_ap(mc))

    project_dr(d["wo1"], lambda k4: Ot[:, 2 * k4:2 * k4 + 2, :], KC, o1_consume,
               wpool, "wo1", proj_psum)
    if KDBG:
        nc.sync.dma_start(d["dbg_ot"], Ot[:])
        for k4 in range(4):
            nc.sync.dma_start(d["dbg_vt"][k4], Vt[k4][:])
        for mc in range(KC):
            nc.sync.dma_start(d["dbg_x1"][mc], x1[mc][:])
    close("resp", "otp")

    ln2 = wpool.tile([P, KC, T], F8, name="ln2", tag="ln2", bufs=1)
    ln_psum = pool("ln_psum2", 1, space="PSUM")
    layernorm(x1, T, 1, ln2, ln_psum)
    close("ln_psum2")

    ep2 = pool("ep2", 1, side="right")
    Q2t = [ep2.tile([P, T], F16, name=f"q2t_{mc}", tag="q2t", bufs=KC)
           for mc in range(KC)]

    def q2_consume(mc, ps):
        if mc % 2 == 0:
            nc.scalar.mul(Q2t[mc][:], ps[:], ws_inv)
        else:
            nc.vector.tensor_scalar_mul(Q2t[mc][:], ps[:], ws_inv)

    project_dr(d["wq2"], lambda k4: ln2[:, 2 * k4:2 * k4 + 2, :], KC, q2_consume,
               wpool, "wq2", proj_psum)
    close("wpool1", "proj_psum2")

    # prefetch first FF weights during attn2
    ffw = pool("ffw", 1)
    ff_pre = []
    for j in range(2):
        wg = ffw.tile([P, KC, P], F16, name=f"pwg_{j}", tag="pwg", bufs=2)
        nc.sync.dma_start(wg[:], d["wff1"][JFF + j])
        wa = ffw.tile([P, KC, P], F16, name=f"pwa_{j}", tag="pwa", bufs=2)
        nc.sync.dma_start(wa[:], d["wff1"][j])
        ff_pre.append((wg, wa))

    # ---------------- phase 5: attn2 (batched) ----------------

    o2p = pool("o2p", 1)
    O2t = o2p.tile([P, KC, T], F8, name="o2t")
    sc2_psum = pool("sc2_psum", 1, space="PSUM")
    ov2_psum = pool("ov2_psum", 1, space="PSUM")
    bp2_psum = pool("bp2_psum", 1, space="PSUM")

    e2 = []
    for c in range(KC):
        sps = sc2_psum.tile([MCTX, 1024], F32, name=f"sp2_{c}", tag="sc2", bufs=2)
        for h in range(2):
            nc.tensor.matmul(sps[:, h * 512:(h + 1) * 512],
                             K2t[c][64 * h:64 * h + 64, :],
                             Q2t[c][64 * h:64 * h + 64, :],
                             start=True, stop=True, tile_position=(64 * h, 0))
        e = ep2.tile([MCTX, 1024], F16, name=f"e2_{c}", tag="e2", bufs=KC)
        nc.scalar.activation(e[:], sps[:], AF.Exp, scale=SCALE)
        e2.append(e)

    def finish2(c, ov):
        usbp = tmp.tile([P, 512], F16, name=f"usb2_{c}", tag="usb", bufs=3)
        dens = [tmp.tile([1, 512], F32, name=f"den2_{c}_{h}", tag=f"den{h}", bufs=2)
                for h in range(2)]
        for h in range(2):
            nc.scalar.copy(usbp[64 * h:64 * h + 64, :], ov[h][0:64, :])
            nc.scalar.copy(dens[h][:], ov[h][64:65, :])
        reca = tmp.tile([1, 512], F32, name=f"rarc16b{c}", tag="reca", bufs=2)
        recb = tmp.tile([1, 512], F32, name=f"rbrc16b{c}", tag="recb", bufs=2)
        nc.vector.reciprocal_approx_fast(reca[:], dens[0][:])
        nc.vector.reciprocal_approx_fast(recb[:], dens[1][:])
        rec16 = tmp.tile([33, 512], F16, name=f"rc16b{c}", tag="rec16", bufs=3)
        nc.vector.memset(rec16[:], 0.0)
        nc.vector.tensor_copy(out=rec16[0:1, :], in_=reca[:])
        nc.vector.tensor_copy(out=rec16[32:33, :], in_=recb[:])
        bps = bp2_psum.tile([P, 512], F32, name=f"bps2_{c}", tag="bps", bufs=1)
        nc.tensor.matmul(bps[:], sel2[:], rec16[:], start=True, stop=True)
        nc.vector.tensor_mul(O2t[:, c, :], usbp[:], bps[:])

    for c in range(KC):
        ov = [ov2_psum.tile([DHEAD + 1, 512], F32, name=f"o2v{2 * c + h}", tag="ov2",
                            bufs=2) for h in range(2)]
        for h in range(2):
            nc.tensor.matmul(ov[h][:], V2t[0:MCTX, 2 * c + h, 0:DHEAD + 1],
                             e2[c][:, h * 512:(h + 1) * 512], start=True, stop=True)
        finish2(c, ov)

    close("ep2", "bp2_psum", "ov2_psum", "sc2_psum")

    # ---------------- phase 6: out-proj 2 + residual, LN3 ----------------

    o2proj_psum = pool("o2proj_psum", 1, space="PSUM")

    def o2_consume(mc, ps):
        nc.vector.scalar_tensor_tensor(x1[mc][:], ps[:], ws_inv, x1[mc][:],
                                       ALU.mult, ALU.add)
        if not trivial_bias:
            nc.vector.tensor_scalar_add(x1[mc][:], x1[mc][:], bias_ap(10 + mc))

    project_dr(d["wo2"], lambda k4: O2t[:, 2 * k4:2 * k4 + 2, :], KC, o2_consume,
               ffw, "wo2", o2proj_psum, wbufs=2)

    ln3p = pool("ln3p", 1)
    ln3 = ln3p.tile([P, KC, T], F16, name="ln3")
    ln_psum = pool("ln_psum3", 1, space="PSUM")
    layernorm(x1, T, 2, ln3, ln_psum)
    close("ln_psum3", "o2proj_psum")

    # ---------------- phase 7: GEGLU FF (fp16) ----------------

    hhp = pool("hhp", 1)
    hht = [hhp.tile([P, T], F16, name=f"hh_{j}", tag="hh", bufs=JFF) for j in range(JFF)]

    proj_psum4 = pool("proj_psum4", 1, space="PSUM")
    for j in range(JFF):
        if j < len(ff_pre):
            wg, wa = ff_pre[j]
        else:
            wg = ffw.tile([P, KC, P], F16, name=f"wg_{j}", tag="wff1g", bufs=2)
            nc.sync.dma_start(wg[:], d["wff1"][JFF + j])
            wa = None
        gps = proj_psum4.tile([P, 512], F32, name=f"gps_{j}", tag="proj", bufs=4)
        for kc in range(KC):
            nc.tensor.matmul(gps[:], wg[:, kc], ln3[:, kc, :], start=(kc == 0),
                             stop=(kc == KC - 1))
        gel = tmp.tile([P, T], F16, name=f"gel_{j}", tag="gel", bufs=3)
        if trivial_bias:
            nc.scalar.activation(gel[:], gps[:], AF.Gelu_apprx_tanh)
        else:
            nc.scalar.activation(gel[:], gps[:], AF.Gelu_apprx_tanh,
                                 bias=bias_ap(60 + j))

        if wa is None:
            wa = ffw.tile([P, KC, P], F16, name=f"wa_{j}", tag="wff1a", bufs=2)
            nc.sync.dma_start(wa[:], d["wff1"][j])
        aps = proj_psum4.tile([P, 512], F32, name=f"aps_{j}", tag="proj", bufs=4)
        for kc in range(KC):
            nc.tensor.matmul(aps[:], wa[:, kc], ln3[:, kc, :], start=(kc == 0),
                             stop=(kc == KC - 1))
        if trivial_bias:
            nc.vector.tensor_mul(hht[j][:], aps[:], gel[:])
        else:
            nc.vector.scalar_tensor_tensor(hht[j][:], aps[:], bias_ap(20 + j), gel[:],
                                           ALU.add, ALU.mult)

    # ---------------- phase 8: FF down-proj + residual -> out ----------------

    outp = pool("outp", 2)
    for mc in range(KC):
        wt = ffw.tile([P, JFF, P], F16, name=f"wff2_{mc}", tag="wff2", bufs=2)
        nc.sync.dma_start(wt[:], d["wff2"][mc])
        ps = proj_psum4.tile([P, 512], F32, name=f"psf2_{mc}", tag="proj", bufs=4)
        for kc in range(JFF):
            nc.tensor.matmul(ps[:], wt[:, kc], hht[kc][:], start=(kc == 0),
                             stop=(kc == JFF - 1))
        ot = outp.tile([P, T], F32, name=f"out_{mc}", tag="out")
        if trivial_bias:
            nc.vector.tensor_add(ot[:], ps[:], x1[mc][:])
        else:
            nc.vector.scalar_tensor_tensor(ot[:], ps[:], bias_ap(100 + mc), x1[mc][:],
                                           ALU.add, ALU.add)
        nc.sync.dma_start(d["out"][mc * P:(mc + 1) * P, :], ot[:])

    close("outp", "proj_psum4", "hhp", "ln3p", "o2p",
          "ffw", "x1p", "qkv2", "ln1p", "tmp", "const")


def _pack_dr_lhst(w, ws):
    """[K, M] f32 -> fp8 [M//128, 128, K//256*2*128] DoubleRow stationary layout:
    out[mc, p, k4*256 + j*128 + m] = w[256*k4 + 128*j + p, 128*mc + m] * ws."""
    K, M = w.shape
    nk4, nmc = K // 256, M // P
    w8 = np.clip(w * ws, -240, 240).astype(E4)
    return np.ascontiguousarray(
        w8.reshape(nk4, 2, P, nmc, P).transpose(3, 2, 0, 1, 4).reshape(nmc, P, -1))


def _pack_dr_rhs(w, ws):
    """[K, N] f32 -> fp8 [K//256, 128, 2, N] DoubleRow moving layout."""
    K, N = w.shape
    nk4 = K // 256
    w8 = np.clip(w * ws, -240, 240).astype(E4)
    return np.ascontiguousarray(w8.reshape(nk4, 2, P, N).transpose(0, 2, 1, 3))


def _lhst_layout(w, n_kc, n_mc):
    """[K, M] f32 -> fp16 [n_mc, 128, n_kc, 128] stationary-operand groups."""
    return np.ascontiguousarray(
        w.reshape(n_kc, P, n_mc, P).transpose(2, 1, 0, 3).astype(np.float16))


def _rhs_layout(w, n_kc):
    """[K, M] f32 -> fp16 [n_kc, 128, M] row-chunk (moving-operand) layout."""
    return np.ascontiguousarray(w.reshape(n_kc, P, -1).astype(np.float16))


_BUILT = {}


def _build(trivial_aff, trivial_bias, ws_inv):
    key = (trivial_aff, trivial_bias, ws_inv)
    if key in _BUILT:
        return _BUILT[key]
    nc = bacc.Bacc("TRN2", target_bir_lowering=False, debug=False, num_devices=N_CORES)
    d = {
        "xt": nc.dram_tensor("xt", [DIM, TKV], F32, kind="ExternalInput").ap(),
        "ctxt": nc.dram_tensor("ctxt", [CTX_DIM, MCTX], F32, kind="ExternalInput").ap(),
        "xres": nc.dram_tensor("xres", [DIM, T], F32, kind="ExternalInput").ap(),
        "wq1": nc.dram_tensor("wq1", [KC, P, K4, 2, P], F8, kind="ExternalInput").ap(),
        "wk1": nc.dram_tensor("wk1", [KC, P, K4, 2, P], F8, kind="ExternalInput").ap(),
        "wv1": nc.dram_tensor("wv1", [K4, P, 2, DIM], F8, kind="ExternalInput").ap(),
        "wo1": nc.dram_tensor("wo1", [KC, P, K4, 2, P], F8, kind="ExternalInput").ap(),
        "wq2": nc.dram_tensor("wq2", [KC, P, K4, 2, P], F8, kind="ExternalInput").ap(),
        "wk2": nc.dram_tensor("wk2", [KC, P, KCX, P], F16, kind="ExternalInput").ap(),
        "wv2": nc.dram_tensor("wv2", [KCX, P, DIM], F16, kind="ExternalInput").ap(),
        "wo2": nc.dram_tensor("wo2", [KC, P, K4, 2, P], F8, kind="ExternalInput").ap(),
        "wff1": nc.dram_tensor("wff1", [2 * JFF, P, KC, P], F16,
                               kind="ExternalInput").ap(),
        "wff2": nc.dram_tensor("wff2", [KC, P, JFF, P], F16, kind="ExternalInput").ap(),
        "out": nc.dram_tensor("out", [DIM, T], F32, kind="ExternalOutput").ap(),
    }
    if os.environ.get("KDBG"):
        d["dbg_ln1"] = nc.dram_tensor("dbg_ln1", [P, KC, TKV], F8, kind="ExternalOutput").ap()
        d["dbg_q"] = nc.dram_tensor("dbg_q", [KC, P, T], F16, kind="ExternalOutput").ap()
        d["dbg_k"] = nc.dram_tensor("dbg_k", [KC, P, TKV], F16, kind="ExternalOutput").ap()
        d["dbg_ot"] = nc.dram_tensor("dbg_ot", [P, KC, T], F8, kind="ExternalOutput").ap()
        d["dbg_x1"] = nc.dram_tensor("dbg_x1", [KC, P, T], F32, kind="ExternalOutput").ap()
        d["dbg_vt"] = nc.dram_tensor("dbg_vt", [4, P, 2, HEADS, 80], F8, kind="ExternalOutput").ap()
        d["dbg_usb"] = nc.dram_tensor("dbg_usb", [P, T], F16, kind="ExternalOutput").ap()
        d["dbg_den"] = nc.dram_tensor("dbg_den", [33, T], F32, kind="ExternalOutput").ap()
        d["dbg_rec"] = nc.dram_tensor("dbg_rec", [33, T], F16, kind="ExternalOutput").ap()
    if not trivial_aff:
        d["aff"] = nc.dram_tensor("aff", [P, 60], F32, kind="ExternalInput").ap()
    if not trivial_bias:
        d["biases"] = nc.dram_tensor("biases", [P, 110], F32, kind="ExternalInput").ap()
    with tile.TileContext(nc) as tc:
        _emit(tc, d, trivial_aff, trivial_bias, ws_inv)
    nc.compile()
    _BUILT[key] = nc
    return nc


def kernel(x, context,
           g1, be1, wq1, wk1, wv1, wo1, bo1,
           g2, be2, wq2, wk2, wv2, wo2, bo2,
           g3, be3, w_ff1, b_ff1, w_ff2, b_ff2,
           _trace=False):
    global last_exec_time_ns
    x = np.asarray(x, np.float32)
    context = np.asarray(context, np.float32)

    affs = [np.asarray(a, np.float32) for a in (g1, be1, g2, be2, g3, be3)]
    biases = [np.asarray(b, np.float32) for b in (bo1, bo2, b_ff1, b_ff2)]
    trivial_aff = all(np.all(a == (1.0 if i % 2 == 0 else 0.0))
                      for i, a in enumerate(affs))
    trivial_bias = all(np.all(b == 0.0) for b in biases)

    w8s = [np.asarray(w, np.float32) for w in (wq1, wk1, wv1, wo1, wq2, wo2)]
    am = max(np.abs(w).max() for w in w8s)
    ws = float(2.0 ** np.floor(np.log2(120.0 / max(am, 1e-30))))
    ws = min(max(ws, 2.0 ** -40), 2.0 ** 40)
    ws_inv = 1.0 / ws

    nc = _build(trivial_aff, trivial_bias, ws_inv)

    shared = {
        "wq1": _pack_dr_lhst(w8s[0], ws),
        "wk1": _pack_dr_lhst(w8s[1], ws),
        "wv1": _pack_dr_rhs(w8s[2], ws),
        "wo1": _pack_dr_lhst(w8s[3], ws),
        "wq2": _pack_dr_lhst(w8s[4], ws),
        "wk2": _lhst_layout(np.asarray(wk2, np.float32), KCX, KC),
        "wv2": _rhs_layout(np.asarray(wv2, np.float32), KCX),
        "wo2": _pack_dr_lhst(w8s[5], ws),
        "wff1": _lhst_layout(np.asarray(w_ff1, np.float32), KC, 2 * JFF),
        "wff2": _lhst_layout(np.asarray(w_ff2, np.float32), JFF, KC),
    }
    if not trivial_aff:
        aff = np.zeros([P, 60], np.float32)
        for i, a in enumerate(affs):
            ln_idx, j = i // 2, i % 2
            aff[:, ln_idx * 20 + j * 10: ln_idx * 20 + j * 10 + 10] = \
                a.reshape(KC, P).T
        shared["aff"] = aff
    if not trivial_bias:
        bb = np.zeros([P, 110], np.float32)
        bb[:, 0:10] = biases[0].reshape(KC, P).T
        bb[:, 10:20] = biases[1].reshape(KC, P).T
        bb[:, 20:100] = biases[2].reshape(2 * JFF, P).T
        bb[:, 100:110] = biases[3].reshape(KC, P).T
        shared["biases"] = bb

    in_maps = []
    for b in range(BATCH):
        ctxt = np.ascontiguousarray(context[b].T)
        for h in range(2):
            xr = np.roll(x[b], -h * T, axis=0)
            m = dict(shared)
            xrt = np.ascontiguousarray(xr.T)
            m["xt"] = xrt
            m["xres"] = np.ascontiguousarray(xrt[:, 0:T])
            m["ctxt"] = ctxt
            in_maps.append(m)

    res = bass_utils.run_bass_kernel_spmd(
        nc, in_maps, core_ids=list(range(N_CORES)), trace=_trace)
    last_exec_time_ns = res.exec_time_ns
    global last_results
    last_results = res.results

    out = np.empty((BATCH, NTOK, DIM), np.float32)
    for b in range(BATCH):
        for h in range(2):
            out[b, h * T:(h + 1) * T, :] = res.results[b * 2 + h]["out"].T
    return out

